# revision 10
# baseline (speedup 1.0000x reference)
"""Trainium2 Bass kernel for nn_EntityEncoder (multi-hot embedding bag + MLP head).

Strategy: vocab (E) sharding across 8 cores. The host lays out each core's
[512, 6250] int32 mask slice in transposed, SBUF-ready form [125, 50*512]
(partition = e-within-subtile, free = subtile-major bp), and the matching
embedding rows as [125, 50*128]. Each core then:
  - DMAs its 12.8 MB mask slice in 5 large transfers and the 3.2 MB embedding
    slice in 1 transfer (large DMAs run near peak HBM bandwidth),
  - casts mask int32 -> bf16 on DVE, embedding f32 -> bf16 on ACT,
  - accumulates sums[h, bp] over 50 K-subtiles of 125 as bf16 matmuls
    straight from the resident tiles (no on-chip transposes),
  - reduces counts with DVE adds + one ones-matmul,
  - AllReduces the [129, 512] partials (sums + counts) across the 8 cores,
  - computes the head (divide, path-mean, LN -> Linear+ReLU -> BN, x2)
    redundantly on every core; the host takes core 0's [32, 128] output.
"""

import numpy as np

B, P, E, H = 32, 16, 50000, 128
NCORES = 8
BP = B * P                 # 512
E_SH = E // NCORES         # 6250 vocab rows per core
SUB = 125                  # matmul K subtile
NSUB = E_SH // SUB         # 50
TILE_SUBS = 10             # subtiles per DMA tile
NTILE = NSUB // TILE_SUBS  # 5
XW = TILE_SUBS * BP        # 5120  x-tile free width
EPS = 1e-5

# packed params layout: [128, NPAR] f32
#  col 0 ln1_g, 1 ln1_b, 2 ln2_g, 3 ln2_b, 4 bn1_g', 5 bn1_b, 6 bn2_g',
#  7 bn2_b, 8 b1, 9 b2, 10:138 w1^T, 138:266 w2^T
NPAR = 266

_cached = {}


def _build():
    import concourse.bacc as bacc
    import concourse.mybir as mybir
    import concourse.tile as tile
    from concourse import masks

    f32 = mybir.dt.float32
    bf16 = mybir.dt.bfloat16
    i32 = mybir.dt.int32

    nc = bacc.Bacc("TRN2", target_bir_lowering=False, debug=False,
                   num_devices=NCORES)

    x_d = nc.dram_tensor("x", [SUB, NSUB * BP], i32, kind="ExternalInput")
    emb_d = nc.dram_tensor("emb", [SUB, NSUB * H], f32, kind="ExternalInput")
    par_d = nc.dram_tensor("par", [128, NPAR], f32, kind="ExternalInput")
    out_d = nc.dram_tensor("out", [B, H], f32, kind="ExternalOutput")

    with tile.TileContext(nc) as tc:
        with tc.tile_pool(name="const", bufs=1) as constp, \
             tc.tile_pool(name="xin", bufs=3) as xin, \
             tc.tile_pool(name="xfp", bufs=2) as xfp, \
             tc.tile_pool(name="head", bufs=1) as head, \
             tc.tile_pool(name="ps_acc", bufs=1, space="PSUM") as ps_acc, \
             tc.tile_pool(name="ps_misc", bufs=3, space="PSUM") as ps_misc, \
             tc.tile_pool(name="dram", bufs=1, space="DRAM") as dram:

            ident = constp.tile([128, 128], f32)
            masks.make_identity(nc, ident[:])
            ones_col = constp.tile([128, 1], f32)
            nc.vector.memset(ones_col[:], 1.0)
            ones_row = constp.tile([1, 128], f32)
            nc.vector.memset(ones_row[:], 1.0)
            zero_1 = constp.tile([1, 1], f32)
            nc.vector.memset(zero_1[:], 0.0)

            par = constp.tile([128, NPAR], f32)
            nc.sync.dma_start(par[:], par_d[:, :])

            # resident embedding: load f32, cast to bf16 on ACT
            emb_f = constp.tile([SUB, NSUB * H], f32)
            nc.sync.dma_start(emb_f[:], emb_d[:, :])
            emb_b = constp.tile([SUB, NSUB * H], bf16)
            nc.scalar.copy(emb_b[:], emb_f[:])

            ones_bf = constp.tile([128, 1], bf16)
            nc.vector.memset(ones_bf[:], 1.0)

            # ---------------- main GEMM loop ----------------
            psum_sums = ps_acc.tile([128, BP], f32)   # [h, bp]
            psum_cnt = ps_acc.tile([1, BP], f32)

            for t in range(NTILE):
                xi = xin.tile([SUB, XW], i32, tag="xi")
                dma_eng = nc.sync if t % 2 == 0 else nc.scalar
                dma_eng.dma_start(xi[:], x_d[:, t * XW:(t + 1) * XW])
                xf = xfp.tile([SUB, XW], bf16, tag="xf")
                cast_eng = nc.vector if t % 2 == 0 else nc.gpsimd
                cast_eng.tensor_copy(xf[:], xi[:])
                for j in range(TILE_SUBS):
                    sidx = t * TILE_SUBS + j
                    nc.tensor.matmul(
                        psum_sums[:],
                        emb_b[:, sidx * H:(sidx + 1) * H],
                        xf[:, j * BP:(j + 1) * BP],
                        start=(sidx == 0), stop=(sidx == NSUB - 1))
                    nc.tensor.matmul(
                        psum_cnt[:],
                        ones_bf[:SUB, :],
                        xf[:, j * BP:(j + 1) * BP],
                        start=(sidx == 0), stop=(sidx == NSUB - 1))

            # ---------------- AllReduce ----------------
            cc_in = dram.tile([129, BP], f32)
            cc_out = dram.tile([129, BP], f32)
            sums_stage = head.tile([128, BP], f32)
            nc.any.tensor_copy(sums_stage[:], psum_sums[:])
            cnt_stage = head.tile([1, BP], f32)
            nc.vector.tensor_copy(cnt_stage[:], psum_cnt[:])
            nc.sync.dma_start(cc_in[0:128, :], sums_stage[:])
            nc.sync.dma_start(cc_in[128:129, :], cnt_stage[:])
            nc.gpsimd.collective_compute(
                "AllReduce",
                mybir.AluOpType.add,
                replica_groups=[list(range(NCORES))],
                ins=[cc_in[:].opt()],
                outs=[cc_out[:].opt()],
            )
            sums_sb = head.tile([128, BP], f32)
            cntT = head.tile([1, BP], f32)
            nc.sync.dma_start(sums_sb[:], cc_out[0:128, :])
            nc.sync.dma_start(cntT[:], cc_out[128:129, :])

            # ---------------- head ----------------
            # path = sums / cnt ; x = sum_p path  (p-mean scale folded into LN1)
            recip = head.tile([1, BP], f32)
            nc.vector.reciprocal(recip[:], cntT[:])
            bc_ps = ps_misc.tile([128, BP], f32, tag="psmisc")
            nc.tensor.matmul(bc_ps[:], ones_row[:], recip[:],
                             start=True, stop=True)
            path = head.tile([128, BP], f32)
            nc.vector.tensor_tensor(out=path[:], in0=sums_sb[:],
                                    in1=bc_ps[:], op=mybir.AluOpType.mult)
            x0 = head.tile([128, B], f32)
            nc.vector.reduce_sum(
                x0[:], path[:].rearrange("h (b p) -> h b p", p=P),
                axis=mybir.AxisListType.X)

            def layer_norm(x_sb, g_col, b_col, eps_val, name):
                sq = head.tile([128, B], f32, tag=f"{name}_sq")
                nc.scalar.square(sq[:], x_sb[:])
                st_ps = ps_misc.tile([1, 2 * B], f32, tag="psmisc")
                nc.tensor.matmul(st_ps[:, 0:B], ones_col[:], x_sb[:],
                                 start=True, stop=True)
                nc.tensor.matmul(st_ps[:, B:2 * B], ones_col[:], sq[:],
                                 start=True, stop=True)
                # mu = Sx/128 ; var+eps = (Sx2/128 + eps) - mu^2
                mr = head.tile([1, 2 * B], f32, tag=f"{name}_mr")
                nc.vector.tensor_scalar(
                    out=mr[:, 0:B], in0=st_ps[:, 0:B],
                    scalar1=1.0 / 128, scalar2=None,
                    op0=mybir.AluOpType.mult)
                mu2 = head.tile([1, B], f32, tag=f"{name}_mu2")
                nc.vector.tensor_tensor(
                    out=mu2[:], in0=mr[:, 0:B], in1=mr[:, 0:B],
                    op=mybir.AluOpType.mult)
                var = head.tile([1, B], f32, tag=f"{name}_var")
                nc.vector.tensor_scalar(
                    out=var[:], in0=st_ps[:, B:2 * B],
                    scalar1=1.0 / 128, scalar2=float(eps_val),
                    op0=mybir.AluOpType.mult, op1=mybir.AluOpType.add)
                nc.vector.tensor_tensor(
                    out=var[:], in0=var[:], in1=mu2[:],
                    op=mybir.AluOpType.subtract)
                sd = head.tile([1, B], f32, tag=f"{name}_sd")
                nc.scalar.activation(sd[:], var[:],
                                     mybir.ActivationFunctionType.Sqrt,
                                     bias=zero_1[:, :1], scale=1.0)
                nc.vector.reciprocal(mr[:, B:2 * B], sd[:])
                # broadcast mu/rstd to all 128 partitions
                bcs = ps_misc.tile([128, 2 * B], f32, tag="psmisc")
                nc.tensor.matmul(bcs[:], ones_row[:], mr[:],
                                 start=True, stop=True)
                xn = head.tile([128, B], f32, tag=f"{name}_xn")
                nc.vector.tensor_tensor(
                    out=xn[:], in0=x_sb[:], in1=bcs[:, 0:B],
                    op=mybir.AluOpType.subtract)
                nc.vector.tensor_tensor(
                    out=xn[:], in0=xn[:], in1=bcs[:, B:2 * B],
                    op=mybir.AluOpType.mult)
                out = head.tile([128, B], f32, tag=f"{name}_out")
                nc.vector.tensor_scalar(
                    out=out[:], in0=xn[:],
                    scalar1=par[:, g_col:g_col + 1],
                    scalar2=par[:, b_col:b_col + 1],
                    op0=mybir.AluOpType.mult, op1=mybir.AluOpType.add)
                return out

            def linear_relu_bn(x_sb, w_lo, b_col, bng_col, bnb_col, name):
                y_ps = ps_misc.tile([128, B], f32, tag="psmisc")
                nc.tensor.matmul(y_ps[:], par[:, w_lo:w_lo + 128], x_sb[:],
                                 start=True, stop=True)
                y = head.tile([128, B], f32, tag=f"{name}_relu")
                nc.scalar.activation(y[:], y_ps[:],
                                     mybir.ActivationFunctionType.Relu,
                                     bias=par[:, b_col:b_col + 1], scale=1.0)
                z = head.tile([128, B], f32, tag=f"{name}_bn")
                nc.vector.tensor_scalar(
                    out=z[:], in0=y[:],
                    scalar1=par[:, bng_col:bng_col + 1],
                    scalar2=par[:, bnb_col:bnb_col + 1],
                    op0=mybir.AluOpType.mult, op1=mybir.AluOpType.add)
                return z

            # LN1 on un-normalized p-sum: eps scales by P^2
            h1 = layer_norm(x0, 0, 1, EPS * P * P, "ln1")
            h2 = linear_relu_bn(h1, 10, 8, 4, 5, "l1")
            h3 = layer_norm(h2, 2, 3, EPS, "ln2")
            h4 = linear_relu_bn(h3, 138, 9, 6, 7, "l2")

            # transpose [128h, 32b] -> [32b, 128h] and store
            out_ps = ps_misc.tile([B, 128], f32, tag="psmisc")
            nc.tensor.transpose(out_ps[:], h4[:], ident[:, :])
            out_sb = head.tile([B, 128], f32)
            nc.vector.tensor_copy(out_sb[:], out_ps[:])
            nc.sync.dma_start(out_d[:, :], out_sb[:])

    nc.compile()
    return nc


def _prepare_in_maps(inputs):
    x = np.asarray(inputs["inputs"])
    emb = np.asarray(inputs["emb"], dtype=np.float32)
    w1 = np.asarray(inputs["w1"], dtype=np.float32)
    b1 = np.asarray(inputs["b1"], dtype=np.float32)
    w2 = np.asarray(inputs["w2"], dtype=np.float32)
    b2 = np.asarray(inputs["b2"], dtype=np.float32)

    par = np.zeros((128, NPAR), dtype=np.float32)
    par[:, 0] = inputs["ln1_g"]
    par[:, 1] = inputs["ln1_b"]
    par[:, 2] = inputs["ln2_g"]
    par[:, 3] = inputs["ln2_b"]
    par[:, 4] = np.asarray(inputs["bn1_g"], np.float32) / np.sqrt(
        np.float32(1.0) + np.float32(EPS))
    par[:, 5] = inputs["bn1_b"]
    par[:, 6] = np.asarray(inputs["bn2_g"], np.float32) / np.sqrt(
        np.float32(1.0) + np.float32(EPS))
    par[:, 7] = inputs["bn2_b"]
    par[:, 8] = b1
    par[:, 9] = b2
    par[:, 10:138] = w1.T
    par[:, 138:266] = w2.T

    x_flat = x.reshape(BP, E)
    in_maps = []
    for c in range(NCORES):
        lo = c * E_SH
        # [bp, e] slice -> [p, j, bp] -> [125, 50*512]  (j = subtile index)
        seg = x_flat[:, lo:lo + E_SH].reshape(BP, NSUB, SUB)
        x_sh = np.ascontiguousarray(
            seg.transpose(2, 1, 0), dtype=np.int32).reshape(SUB, NSUB * BP)
        # emb rows -> [p, j, h] -> [125, 50*128]
        seg_e = emb[lo:lo + E_SH, :]
        if c == 0:
            seg_e = seg_e.copy()
            seg_e[0, :] = 0.0   # padding_idx=0
        emb_sh = np.ascontiguousarray(
            seg_e.reshape(NSUB, SUB, H).transpose(1, 0, 2)
        ).reshape(SUB, NSUB * H)
        in_maps.append({"x": x_sh, "emb": emb_sh, "par": par})
    return in_maps


def _run(inputs, trace=False):
    from concourse.bass_utils import run_bass_kernel_spmd

    if "nc" not in _cached:
        _cached["nc"] = _build()
    nc = _cached["nc"]
    in_maps = _prepare_in_maps(inputs)
    res = run_bass_kernel_spmd(
        nc, in_maps, core_ids=list(range(NCORES)), trace=trace)
    out = np.asarray(res.results[0]["out"])
    return out, res.exec_time_ns


def kernel(**inputs) -> np.ndarray:
    out, _ = _run(inputs, trace=False)
    return out


# revision 11
# speedup vs baseline: 1.5271x; 1.5271x over previous
"""Trainium2 Bass kernel for nn_EntityEncoder (multi-hot embedding bag + MLP head).

Strategy: vocab (E) sharding across 8 cores. The host lays out each core's
[512, 6250] int32 mask slice in transposed, SBUF-ready form [125, 50*512]
(partition = e-within-subtile, free = subtile-major bp), and the matching
embedding rows as [125, 50*128]. Each core then:
  - DMAs its 12.8 MB mask slice in 5 large transfers and the 3.2 MB embedding
    slice in 1 transfer (large DMAs run near peak HBM bandwidth),
  - casts mask int32 -> bf16 on DVE, embedding f32 -> bf16 on ACT,
  - accumulates sums[h, bp] over 50 K-subtiles of 125 as bf16 matmuls
    straight from the resident tiles (no on-chip transposes),
  - reduces counts with DVE adds + one ones-matmul,
  - AllReduces the [129, 512] partials (sums + counts) across the 8 cores,
  - computes the head (divide, path-mean, LN -> Linear+ReLU -> BN, x2)
    redundantly on every core; the host takes core 0's [32, 128] output.
"""

import numpy as np

B, P, E, H = 32, 16, 50000, 128
NCORES = 8
BP = B * P                 # 512
E_SH = E // NCORES         # 6250 vocab rows per core
SUB = 128                  # matmul K subtile (shard zero-padded to 6400)
E_PAD = 6400               # padded vocab rows per core
NSUB = E_PAD // SUB        # 50
TILE_SUBS = 10             # subtiles per DMA tile
NTILE = NSUB // TILE_SUBS  # 5
XW = TILE_SUBS * BP        # 5120  x-tile free width
EPS = 1e-5

# packed params layout: [128, NPAR] f32
#  col 0 ln1_g, 1 ln1_b, 2 ln2_g, 3 ln2_b, 4 bn1_g', 5 bn1_b, 6 bn2_g',
#  7 bn2_b, 8 b1, 9 b2, 10:138 w1^T, 138:266 w2^T
NPAR = 266

_cached = {}


def _build():
    import concourse.bacc as bacc
    import concourse.mybir as mybir
    import concourse.tile as tile
    from concourse import masks

    f32 = mybir.dt.float32
    bf16 = mybir.dt.bfloat16
    i32 = mybir.dt.int32

    nc = bacc.Bacc("TRN2", target_bir_lowering=False, debug=False,
                   num_devices=NCORES)

    x_d = nc.dram_tensor("x", [SUB, NSUB * BP], i32, kind="ExternalInput")
    emb_d = nc.dram_tensor("emb", [SUB, NSUB * H], f32, kind="ExternalInput")
    par_d = nc.dram_tensor("par", [128, NPAR], f32, kind="ExternalInput")
    out_d = nc.dram_tensor("out", [B, H], f32, kind="ExternalOutput")

    with tile.TileContext(nc) as tc:
        with tc.tile_pool(name="const", bufs=1) as constp, \
             tc.tile_pool(name="xin", bufs=3) as xin, \
             tc.tile_pool(name="xfp", bufs=2) as xfp, \
             tc.tile_pool(name="head", bufs=1) as head, \
             tc.tile_pool(name="ps_acc", bufs=1, space="PSUM") as ps_acc, \
             tc.tile_pool(name="ps_misc", bufs=3, space="PSUM") as ps_misc, \
             tc.tile_pool(name="dram", bufs=1, space="DRAM") as dram:

            ident = constp.tile([128, 128], f32)
            masks.make_identity(nc, ident[:])
            ones_col = constp.tile([128, 1], f32)
            nc.vector.memset(ones_col[:], 1.0)
            ones_row = constp.tile([1, 128], f32)
            nc.vector.memset(ones_row[:], 1.0)
            zero_1 = constp.tile([1, 1], f32)
            nc.vector.memset(zero_1[:], 0.0)

            par = constp.tile([128, NPAR], f32)
            nc.sync.dma_start(par[:], par_d[:, :])

            # resident embedding: load f32, cast to bf16 on ACT in chunks
            emb_f = constp.tile([SUB, NSUB * H], f32)
            emb_b = constp.tile([SUB, NSUB * H], bf16)
            EC = NSUB * H // 5
            for k in range(5):
                nc.sync.dma_start(emb_f[:, k * EC:(k + 1) * EC],
                                  emb_d[:, k * EC:(k + 1) * EC])
                nc.scalar.copy(emb_b[:, k * EC:(k + 1) * EC],
                               emb_f[:, k * EC:(k + 1) * EC])

            ones_bf = constp.tile([128, 1], bf16)
            nc.vector.memset(ones_bf[:], 1.0)

            # ---------------- main GEMM loop ----------------
            psum_sums = ps_acc.tile([128, BP], f32)   # [h, bp]
            psum_cnt = ps_acc.tile([1, BP], f32)

            for t in range(NTILE):
                xi = xin.tile([SUB, XW], i32, tag="xi")
                dma_eng = nc.sync if t % 2 == 0 else nc.scalar
                dma_eng.dma_start(xi[:], x_d[:, t * XW:(t + 1) * XW])
                xf = xfp.tile([SUB, XW], bf16, tag="xf")
                cast_eng = nc.gpsimd if t == 1 else nc.vector
                cast_eng.tensor_copy(xf[:], xi[:])
                for j in range(TILE_SUBS):
                    sidx = t * TILE_SUBS + j
                    nc.tensor.matmul(
                        psum_sums[:],
                        emb_b[:, sidx * H:(sidx + 1) * H],
                        xf[:, j * BP:(j + 1) * BP],
                        start=(sidx == 0), stop=(sidx == NSUB - 1))
                    nc.tensor.matmul(
                        psum_cnt[:],
                        ones_bf[:, :],
                        xf[:, j * BP:(j + 1) * BP],
                        start=(sidx == 0), stop=(sidx == NSUB - 1))

            # ---------------- AllReduce ----------------
            cc_in = dram.tile([129, BP], f32)
            cc_out = dram.tile([129, BP], f32)
            sums_stage = head.tile([128, BP], f32)
            nc.any.tensor_copy(sums_stage[:], psum_sums[:])
            cnt_stage = head.tile([1, BP], f32)
            nc.vector.tensor_copy(cnt_stage[:], psum_cnt[:])
            nc.sync.dma_start(cc_in[0:128, :], sums_stage[:])
            nc.sync.dma_start(cc_in[128:129, :], cnt_stage[:])
            nc.gpsimd.collective_compute(
                "AllReduce",
                mybir.AluOpType.add,
                replica_groups=[list(range(NCORES))],
                ins=[cc_in[:].opt()],
                outs=[cc_out[:].opt()],
            )
            sums_sb = head.tile([128, BP], f32)
            cntT = head.tile([1, BP], f32)
            nc.sync.dma_start(sums_sb[:], cc_out[0:128, :])
            nc.sync.dma_start(cntT[:], cc_out[128:129, :])

            # ---------------- head ----------------
            # path = sums / cnt ; x = sum_p path  (p-mean scale folded into LN1)
            recip = head.tile([1, BP], f32)
            nc.vector.reciprocal(recip[:], cntT[:])
            bc_ps = ps_misc.tile([128, BP], f32, tag="psmisc")
            nc.tensor.matmul(bc_ps[:], ones_row[:], recip[:],
                             start=True, stop=True)
            path = head.tile([128, BP], f32)
            nc.vector.tensor_tensor(out=path[:], in0=sums_sb[:],
                                    in1=bc_ps[:], op=mybir.AluOpType.mult)
            x0 = head.tile([128, B], f32)
            nc.vector.reduce_sum(
                x0[:], path[:].rearrange("h (b p) -> h b p", p=P),
                axis=mybir.AxisListType.X)

            def layer_norm(x_sb, g_col, b_col, eps_val, name):
                sq = head.tile([128, B], f32, tag=f"{name}_sq")
                nc.scalar.square(sq[:], x_sb[:])
                st_ps = ps_misc.tile([1, 2 * B], f32, tag="psmisc")
                nc.tensor.matmul(st_ps[:, 0:B], ones_col[:], x_sb[:],
                                 start=True, stop=True)
                nc.tensor.matmul(st_ps[:, B:2 * B], ones_col[:], sq[:],
                                 start=True, stop=True)
                # mu = Sx/128 ; var+eps = (Sx2/128 + eps) - mu^2
                mr = head.tile([1, 2 * B], f32, tag=f"{name}_mr")
                nc.vector.tensor_scalar(
                    out=mr[:, 0:B], in0=st_ps[:, 0:B],
                    scalar1=1.0 / 128, scalar2=None,
                    op0=mybir.AluOpType.mult)
                mu2 = head.tile([1, B], f32, tag=f"{name}_mu2")
                nc.vector.tensor_tensor(
                    out=mu2[:], in0=mr[:, 0:B], in1=mr[:, 0:B],
                    op=mybir.AluOpType.mult)
                var = head.tile([1, B], f32, tag=f"{name}_var")
                nc.vector.tensor_scalar(
                    out=var[:], in0=st_ps[:, B:2 * B],
                    scalar1=1.0 / 128, scalar2=float(eps_val),
                    op0=mybir.AluOpType.mult, op1=mybir.AluOpType.add)
                nc.vector.tensor_tensor(
                    out=var[:], in0=var[:], in1=mu2[:],
                    op=mybir.AluOpType.subtract)
                sd = head.tile([1, B], f32, tag=f"{name}_sd")
                nc.scalar.activation(sd[:], var[:],
                                     mybir.ActivationFunctionType.Sqrt,
                                     bias=zero_1[:, :1], scale=1.0)
                nc.vector.reciprocal(mr[:, B:2 * B], sd[:])
                # broadcast mu/rstd to all 128 partitions
                bcs = ps_misc.tile([128, 2 * B], f32, tag="psmisc")
                nc.tensor.matmul(bcs[:], ones_row[:], mr[:],
                                 start=True, stop=True)
                xn = head.tile([128, B], f32, tag=f"{name}_xn")
                nc.vector.tensor_tensor(
                    out=xn[:], in0=x_sb[:], in1=bcs[:, 0:B],
                    op=mybir.AluOpType.subtract)
                nc.vector.tensor_tensor(
                    out=xn[:], in0=xn[:], in1=bcs[:, B:2 * B],
                    op=mybir.AluOpType.mult)
                out = head.tile([128, B], f32, tag=f"{name}_out")
                nc.vector.tensor_scalar(
                    out=out[:], in0=xn[:],
                    scalar1=par[:, g_col:g_col + 1],
                    scalar2=par[:, b_col:b_col + 1],
                    op0=mybir.AluOpType.mult, op1=mybir.AluOpType.add)
                return out

            def linear_relu_bn(x_sb, w_lo, b_col, bng_col, bnb_col, name):
                y_ps = ps_misc.tile([128, B], f32, tag="psmisc")
                nc.tensor.matmul(y_ps[:], par[:, w_lo:w_lo + 128], x_sb[:],
                                 start=True, stop=True)
                y = head.tile([128, B], f32, tag=f"{name}_relu")
                nc.vector.tensor_scalar(
                    out=y[:], in0=y_ps[:],
                    scalar1=par[:, b_col:b_col + 1], scalar2=0.0,
                    op0=mybir.AluOpType.add, op1=mybir.AluOpType.max)
                z = head.tile([128, B], f32, tag=f"{name}_bn")
                nc.vector.tensor_scalar(
                    out=z[:], in0=y[:],
                    scalar1=par[:, bng_col:bng_col + 1],
                    scalar2=par[:, bnb_col:bnb_col + 1],
                    op0=mybir.AluOpType.mult, op1=mybir.AluOpType.add)
                return z

            # LN1 on un-normalized p-sum: eps scales by P^2
            h1 = layer_norm(x0, 0, 1, EPS * P * P, "ln1")
            h2 = linear_relu_bn(h1, 10, 8, 4, 5, "l1")
            h3 = layer_norm(h2, 2, 3, EPS, "ln2")
            h4 = linear_relu_bn(h3, 138, 9, 6, 7, "l2")

            # transpose [128h, 32b] -> [32b, 128h] and store
            out_ps = ps_misc.tile([B, 128], f32, tag="psmisc")
            nc.tensor.transpose(out_ps[:], h4[:], ident[:, :])
            out_sb = head.tile([B, 128], f32)
            nc.vector.tensor_copy(out_sb[:], out_ps[:])
            nc.sync.dma_start(out_d[:, :], out_sb[:])

    nc.compile()
    return nc


def _prepare_in_maps(inputs):
    x = np.asarray(inputs["inputs"])
    emb = np.asarray(inputs["emb"], dtype=np.float32)
    w1 = np.asarray(inputs["w1"], dtype=np.float32)
    b1 = np.asarray(inputs["b1"], dtype=np.float32)
    w2 = np.asarray(inputs["w2"], dtype=np.float32)
    b2 = np.asarray(inputs["b2"], dtype=np.float32)

    par = np.zeros((128, NPAR), dtype=np.float32)
    par[:, 0] = inputs["ln1_g"]
    par[:, 1] = inputs["ln1_b"]
    par[:, 2] = inputs["ln2_g"]
    par[:, 3] = inputs["ln2_b"]
    par[:, 4] = np.asarray(inputs["bn1_g"], np.float32) / np.sqrt(
        np.float32(1.0) + np.float32(EPS))
    par[:, 5] = inputs["bn1_b"]
    par[:, 6] = np.asarray(inputs["bn2_g"], np.float32) / np.sqrt(
        np.float32(1.0) + np.float32(EPS))
    par[:, 7] = inputs["bn2_b"]
    par[:, 8] = b1
    par[:, 9] = b2
    par[:, 10:138] = w1.T
    par[:, 138:266] = w2.T

    x_flat = x.reshape(BP, E)
    in_maps = []
    for c in range(NCORES):
        lo = c * E_SH
        # [bp, e] slice -> pad e to 6400 -> [p, j, bp] -> [128, 50*512]
        seg_t = np.zeros((E_PAD, BP), dtype=np.int32)
        seg_t[:E_SH] = x_flat[:, lo:lo + E_SH].T
        x_sh = np.ascontiguousarray(
            seg_t.reshape(NSUB, SUB, BP).transpose(1, 0, 2)
        ).reshape(SUB, NSUB * BP)
        # emb rows -> pad -> [p, j, h] -> [128, 50*128]
        seg_e = np.zeros((E_PAD, H), dtype=np.float32)
        seg_e[:E_SH] = emb[lo:lo + E_SH, :]
        if c == 0:
            seg_e[0, :] = 0.0   # padding_idx=0
        emb_sh = np.ascontiguousarray(
            seg_e.reshape(NSUB, SUB, H).transpose(1, 0, 2)
        ).reshape(SUB, NSUB * H)
        in_maps.append({"x": x_sh, "emb": emb_sh, "par": par})
    return in_maps


def _run(inputs, trace=False):
    from concourse.bass_utils import run_bass_kernel_spmd

    if "nc" not in _cached:
        _cached["nc"] = _build()
    nc = _cached["nc"]
    in_maps = _prepare_in_maps(inputs)
    res = run_bass_kernel_spmd(
        nc, in_maps, core_ids=list(range(NCORES)), trace=trace)
    out = np.asarray(res.results[0]["out"])
    return out, res.exec_time_ns


def kernel(**inputs) -> np.ndarray:
    out, _ = _run(inputs, trace=False)
    return out
